# revision 57
# baseline (speedup 1.0000x reference)
"""Multi-head self-attention (B=2, T=2048, d_model=1024, 16 heads, causal)
on 8 trn2 NeuronCores.

Sharding: core c -> batch b=c//4, head-group g=c%4 (4 heads, d_model slice
of 256). Each core computes its heads' attention and a partial wo
projection [2048, 1024] (bf16); host sums the 4 partials per batch and
adds bo + bv @ wo^T (the V-bias commutes through softmax since rows sum
to 1, so it is a host-side constant).

Per-core pipeline (all matmul inputs bf16, fp32 PSUM accumulation):
  QT[dq,t] = (wq_s @ x^T)*0.125 + bq*0.125   (scale folded into weights)
  KT[dk,t] = wk_s @ x^T + bk
  V[t,dv]  = x @ wv_s^T
  per head pair, per q-block j (512 wide):
    S^T[k,q] = K_h @ Q_h^T   (K=64 contraction, heads packed at partition
                              bases 0/64 -> concurrent row-group matmuls)
    E = exp(S^T)             (ACT, one call per (head-pair, ktile))
    diag tiles: E *= upper-tri mask
    O^T_aug = V_aug^T @ E    (V_aug = [64 ones cols | V_h cols] so PSUM rows
                              0-63 = replicated rowsums, rows 64-127 = O^T)
    OT_all = O^T * recip(rowsum)  (DVE reciprocal + tensor_tensor mul)
  P = OT_all^T @ wo_s^T      (partial output, bf16 out)

Scheduling: the per-ktile chain S(PE) -> exp(ACT) -> PV(PE) leaves PE
under-filled during attention (ACT needs ~1.15us per ktile vs ~0.64us of
attention PE work).  Projection and wo matmuls for neighboring q-blocks
are emitted as generator-based filler chunks (~0.4us of PE work per
chunk) interleaved between attention ktile steps; wo work is held back
to round 3, the most ACT-bound stretch.  x is staged t-slice-major
(both DRAM and SBUF) so each input DMA moves 8KB-contiguous lines per
partition at full rate and projections of t-slice 0 start ~10us in.
"""
import sys
sys.path.insert(0, "/opt/trn_rl_repo")
from collections import deque

import numpy as np
import ml_dtypes

import concourse.bass as bass
import concourse.bacc as bacc
import concourse.tile as tile
import concourse.mybir as mybir
from concourse import bass_utils

BF16 = mybir.dt.bfloat16
F32 = mybir.dt.float32
F8 = mybir.dt.float8e4
EXP = mybir.ActivationFunctionType.Exp
DROW = mybir.MatmulPerfMode.DoubleRow

T = 2048          # sequence length
DM = 1024         # d_model
DS = 256          # per-core d_model slice (4 heads x 64)
HD = 64           # head dim
NH = 4            # heads per core
KT128 = 16        # k tiles of 128 over T
QB = 512          # q block width
NJ = T // QB      # 4 q blocks
NCORES = 8
# Scores are pre-scaled by 1/SSCL (folded into wq with the 1/sqrt(64)):
# ACT computes k*exp(score) via exp(t*SSCL + LNK); the DVE EXP4 custom op
# computes the same k*exp(score) as (((t+FA)t+FB)((t+FC)t+1))^2.  The
# common factor k cancels in the softmax normalization.
SSCL = 5.011862351873014
FA = 0.5023871747892565
FB = 0.8155188024921279
FC = 1.9177204582091354
LNK = -0.3906937361657684
WSCL_Q = 1024.0   # fp8 pre-scale on wq.T*0.125/SSCL (std 5e-4 -> 0.51)
WSCL_K = 32.0     # fp8 pre-scale on wk.T (std 0.02 -> 0.64)
QDSC = 1.0 / WSCL_Q
KDSC = 1.0 / WSCL_K


def _register_exp4():
    from concourse import dve_ops as DO
    from concourse.dve_spec import Spec, Src0, C0, C1, C2, One, sq, lower
    from concourse.dve_uop import DveOpSpec

    if "EXP4_ANT" in DO._SUB_OPCODE_FOR_NAME:
        return DO.OPS[DO._SUB_OPCODE_FOR_NAME["EXP4_ANT"]
                      - DO._CUSTOM_DVE_ROW_BASE]
    x = Src0
    body = sq(((x + C0) * x + C1) * ((x + C2) * x + One))

    def ref(in0, in1, s0, s1, imm2):
        xr = in0.astype(np.float32)
        p = ((xr + s0) * xr + s1) * ((xr + imm2) * xr + 1.0)
        return (p * p).astype(np.float32)

    spec = Spec(body=body, reference=ref)
    row = DO._CUSTOM_DVE_ROW_BASE + len(DO.OPS)
    shas = {}
    for ver in ("v3", "v4"):
        uops = lower(spec, ver=ver)
        shas[ver] = DveOpSpec(name="EXP4_ANT", opcode=row, uops=uops,
                              rd1_en=False).sha(ver)
    op = DO.DveOp("EXP4_ANT", spec, subdim=False, uops_sha=shas)
    DO.OPS.append(op)
    DO._SUB_OPCODE_FOR_NAME["EXP4_ANT"] = row
    DO.CUSTOM_DVE_SPECS["EXP4_ANT"] = spec
    return op


EXP4 = _register_exp4()

_CACHE = {}


def _build():
    nc = bacc.Bacc("TRN2", target_bir_lowering=False, debug=False,
                   enable_asserts=False, num_devices=NCORES)
    dram = {}
    for name, shape, dt in [
        ("xt", [NJ, 128, 8, 512], BF16),  # x[b]^T, t-slice-major chunks
        ("xt8", [NJ, 128, 8, 512], F8),   # same, fp8 (QK-proj rhs)
        ("wqt", [DM, DS], F8),        # wq.T[:, slice] * 0.125/SSCL * 1024
        ("wkt", [DM, DS], F8),        # wk.T[:, slice] * 32 (fp8)
        ("wvt", [DM, DS], BF16),
        ("wot", [DS, DM], BF16),      # wo[:, slice].T
        ("bqc", [128, 2], F32),       # bq*0.125 as [128, m] columns
        ("bkc", [128, 2], F32),
        ("tri", [128, 128], BF16),    # upper-tri (incl diag) ones
    ]:
        dram[name] = nc.dram_tensor(name, shape, dt, kind="ExternalInput").ap()
    p_out = nc.dram_tensor("p_out", [T, DM], BF16, kind="ExternalOutput").ap()

    with tile.TileContext(nc) as tc:
        with tc.tile_pool(name="persist", bufs=1) as pp, \
             tc.tile_pool(name="epool", bufs=4) as ep, \
             tc.tile_pool(name="outp", bufs=2) as op, \
             tc.tile_pool(name="bcp", bufs=2) as bp, \
             tc.tile_pool(name="misc_ps", bufs=2, space="PSUM") as mp, \
             tc.tile_pool(name="st_ps", bufs=2, space="PSUM") as sp, \
             tc.tile_pool(name="ot_ps", bufs=1, space="PSUM") as tp:

            # ---- persistent SBUF ----
            xt = pp.tile([128, NJ, 8, 512], BF16, name="xt")  # [p, ts, kt8, c]
            xt8 = pp.tile([128, NJ, 8, 512], F8, name="xt8")  # fp8 copy (QK)
            wqt = pp.tile([128, 8, DS], F8, name="wqt")
            wkt = pp.tile([128, 8, DS], F8, name="wkt")
            wvt = pp.tile([128, 8, DS], BF16, name="wvt")
            wot = pp.tile([128, 2, DM], BF16, name="wot")
            qt = pp.tile([128, 2, T], BF16, name="qt")        # [p, dq-tile, t]
            kt = pp.tile([128, 2, T], BF16, name="kt")
            vaug = pp.tile([128, KT128, 512], BF16, name="vaug")
            vaug8 = pp.tile([128, KT128, 512], F8, name="vaug8")
            ot_all = pp.tile([128, 2, T], BF16, name="ot_all")
            wo3pb = pp.tile([128, 4, DM], BF16, name="wo3pb")
            ones_row = pp.tile([1, 512], BF16, name="ones_row")
            bqc = pp.tile([128, 2], F32, name="bqc")
            bkc = pp.tile([128, 2], F32, name="bkc")
            tri = pp.tile([128, 128], BF16, name="tri")
            lnkc = pp.tile([128, 1], F32, name="lnkc")

            # ones_row feeds the PE warmup; DVE is otherwise idle at start.
            nc.vector.memset(ones_row, 1.0)
            nc.vector.memset(lnkc, LNK)
            # PE warmup: dummy matmuls on scratch during the input DMA wait so
            # HAM is at full clock when real matmuls start (results unread)
            warm = mp.tile([128, 512], F32, name="warm", tag="mp")
            # bridges the input-DMA window at the throttled clock so HAM
            # never sees a >3.4us PE idle and proj(0) starts at full clock
            for _ in range(8):
                nc.tensor.matmul(warm, lhsT=ones_row[0:1, 0:128],
                                 rhs=ones_row[0:1, :], start=True, stop=True)

            # Input DMA. A single queue sustains only ~100GB/s, so the head
            # load is split in priority order across the three trigger-capable
            # engines (sync/scalar/gpsimd), in chunks small enough that the
            # proj(0) contraction can start while the rest streams in.
            def w_dram(nm):
                return dram[nm].rearrange("(kt p) d -> p kt d", p=128)

            def w_dram_half(nm, m):
                return dram[nm].rearrange(
                    "(kt p) d -> p kt d", p=128)[:, :, 128 * m:128 * (m + 1)]

            wq_d, wk_d, wv_d, wo_d = (w_dram(n) for n in
                                      ("wqt", "wkt", "wvt", "wot"))
            # Everything needed in the first ~25us rides the fast sync
            # (HWDGE) queue in need-order -- the scalar/gpsimd SWDGE queues
            # only sustain ~30GB/s, so they carry the late-needed bulk:
            # x_2 (attention(2) diag), x8_3/x_3/wot (round 3).
            # The DMA queue round-robins packets across every queued
            # transfer, so completion order ~= trigger order only when a
            # queue is lightly loaded.  Critical path (wqt -> xt8_0 -> x_0)
            # leads sync; tiny-row bqc/bkc (8B rows, trigger-dominated)
            # lead scalar; late-needed bulk rides gpsimd.
            nc.sync.dma_start(out=wqt[:, :, 0:128], in_=w_dram_half("wqt", 0))
            nc.sync.dma_start(out=xt8[:, 0, 0:4], in_=dram["xt8"][0, :, 0:4])
            nc.sync.dma_start(out=wqt[:, :, 128:256], in_=w_dram_half("wqt", 1))
            nc.sync.dma_start(out=xt8[:, 0, 4:8], in_=dram["xt8"][0, :, 4:8])
            nc.sync.dma_start(out=xt[:, 0, 0:4], in_=dram["xt"][0, :, 0:4])
            nc.sync.dma_start(out=xt[:, 0, 4:8], in_=dram["xt"][0, :, 4:8])
            nc.sync.dma_start(out=wvt, in_=wv_d)
            nc.sync.dma_start(
                out=xt8[:, 1:4],
                in_=dram["xt8"][1:4].rearrange("ts p kt c -> p ts kt c"))
            nc.sync.dma_start(
                out=xt[:, 1:4],
                in_=dram["xt"][1:4].rearrange("ts p kt c -> p ts kt c"))
            nc.scalar.dma_start(out=bqc, in_=dram["bqc"])
            nc.scalar.dma_start(out=bkc, in_=dram["bkc"])
            nc.scalar.dma_start(out=wkt, in_=wk_d)
            nc.scalar.dma_start(out=tri, in_=dram["tri"])
            nc.gpsimd.dma_start(out=wot, in_=wo_d)
            # V_aug head block h: cols [128h, 128h+64) ones, [128h+64, +128) V.
            # After the DMA triggers so they don't delay the weight loads.
            for h in range(NH):
                nc.gpsimd.memset(vaug[:, :, 128 * h:128 * h + HD], 1.0)
                nc.gpsimd.memset(vaug8[:, :, 128 * h:128 * h + HD], 1.0)

            # ---- emission helpers ----
            def proj_qk_m(w_sb, b_c, dst, ts, m, descale, chunk=99):
                """One [128,512] psum tile of the Q/K projection (fp8
                DoubleRow, 2 k-subtiles per matmul); yields every `chunk`
                matmuls. The fp8 weight pre-scale is undone via `descale`
                in the bias stage."""
                t0 = ts * 512
                ps = mp.tile([128, 512], F32, name="proj_ps", tag="mp")
                for k in range(4):
                    nc.tensor.matmul(
                        ps, lhsT=w_sb[:, 2 * k:2 * k + 2,
                                      m * 128:(m + 1) * 128],
                        rhs=xt8[:, ts, 2 * k:2 * k + 2, :],
                        start=(k == 0), stop=(k == 3), perf_mode=DROW)
                    if k % chunk == chunk - 1 and k < 3:
                        yield
                if ts < 3:
                    # ACT has slack in the early rounds and reads PSUM fast;
                    # this also frees the psum slot sooner than the busier DVE
                    nc.scalar.activation(
                        out=dst[:, m, t0:t0 + 512], in_=ps,
                        func=mybir.ActivationFunctionType.Identity,
                        bias=b_c[:, m:m + 1], scale=descale)
                else:
                    nc.vector.tensor_scalar(
                        dst[:, m, t0:t0 + 512], ps, descale,
                        b_c[:, m:m + 1], mybir.AluOpType.mult,
                        mybir.AluOpType.add)
                yield

            def proj_v_tt(g):
                """V projection for t-subtile g (128 rows): 8 matmuls + the
                scatter into vaug."""
                ps = mp.tile([128, 256], F32, name="v_ps", tag="mp")
                for k in range(8):
                    nc.tensor.matmul(
                        ps, lhsT=xt[:, g // 4, k, (g % 4) * 128:(g % 4 + 1) * 128],
                        rhs=wvt[:, k, :], start=(k == 0), stop=(k == 7))
                    if k == 3:
                        yield
                # scatter into vaug (bf16 diag path + fp8 off-diag path):
                # head h -> cols [128h+64, 128h+128)
                nc.vector.tensor_copy(
                    vaug[:, g, :].rearrange("p (h c) -> p h c", h=NH)[:, :, HD:],
                    ps.rearrange("p (h c) -> p h c", h=NH))
                # the fp8 copy derives from the bf16 one in SBUF on ACT
                # (gpsimd can't read PSUM and queues ahead of diag masks;
                # DVE is the most loaded engine)
                nc.scalar.activation(
                    out=vaug8[:, g, :].rearrange(
                        "p (h c) -> p h c", h=NH)[:, :, HD:],
                    in_=vaug[:, g, :].rearrange(
                        "p (h c) -> p h c", h=NH)[:, :, HD:],
                    func=mybir.ActivationFunctionType.Copy)
                yield

            def gen_proj_qk(ts):
                for w_sb, b_c, dst, dsc in ((wqt, bqc, qt, QDSC),
                                            (wkt, bkc, kt, KDSC)):
                    for m in range(2):
                        yield from proj_qk_m(w_sb, b_c, dst, ts, m, dsc,
                                             chunk=1)

            def gen_proj_v(ts):
                for tt in range(4):
                    yield from proj_v_tt(4 * ts + tt)

            def gen_wo(j):
                q0 = j * QB
                last = j == NJ - 1
                ob = op.tile([128, 4, DM], BF16, name="ob", tag="ob")
                for qq in range(4):
                    row = q0 + qq * 128
                    for n in range(2):
                        ps = mp.tile([128, 512], F32, name="wo_ps", tag="mp")
                        for kk in range(2):
                            nc.tensor.matmul(
                                ps, lhsT=ot_all[:, kk, row:row + 128],
                                rhs=wot[:, kk, n * 512:(n + 1) * 512],
                                start=(kk == 0), stop=(kk == 1))
                        # split the psum->sbuf casts between DVE and ACT so
                        # neither engine stalls the attention exp chain
                        if n == 0:
                            nc.vector.tensor_copy(
                                ob[:, qq, n * 512:(n + 1) * 512], ps)
                        else:
                            nc.scalar.activation(
                                out=ob[:, qq, n * 512:(n + 1) * 512], in_=ps,
                                func=mybir.ActivationFunctionType.Identity)
                        yield
                    if last:
                        # tail round: stream the output per 128-row chunk so
                        # the final DMA only trails the last copy (HWDGE
                        # queues only -- gpsimd's SWDGE has slow start)
                        dma_eng = nc.sync if qq % 2 == 0 else nc.scalar
                        dma_eng.dma_start(
                            out=p_out[row:row + 128, :], in_=ob[:, qq, :])
                if not last:
                    dma_eng = nc.sync if j % 2 == 0 else nc.scalar
                    dma_eng.dma_start(
                        out=p_out[q0:q0 + 512, :].rearrange(
                            "(q p) d -> p q d", p=128),
                        in_=ob)
                yield

            # Global filler queue: (key, generator), drained strictly FIFO.
            fillers = deque()

            def fill(n):
                while n > 0 and fillers:
                    try:
                        next(fillers[0][1])
                        n -= 1
                    except StopIteration:
                        fillers.popleft()

            def force(key):
                """Drain fillers until generator `key` has been exhausted."""
                while any(k == key for k, _ in fillers):
                    try:
                        next(fillers[0][1])
                    except StopIteration:
                        fillers.popleft()

            def attention(j, fill_fn, end_h=None):
                q0 = j * QB
                nk = 4 * (j + 1)           # k-tiles of 128
                for H in range(2):          # head pair (2H, 2H+1)
                    ot = [tp.tile([128, QB], F32, name=f"ot{hp}", tag=f"ot{hp}")
                          for hp in range(2)]

                    def pv_diag(ktile, c0, e_t):
                        for hp in range(2):
                            h = 2 * H + hp
                            nc.tensor.matmul(
                                ot[hp][:, c0:QB],
                                lhsT=vaug[:, ktile, 128 * h:128 * (h + 1)],
                                rhs=e_t[:, hp, c0:QB],
                                start=(ktile == 0), stop=(ktile == nk - 1))

                    def pv_pair(tpi, e8t):
                        # fp8 DoubleRow: ktiles (2tpi, 2tpi+1) in one matmul
                        # per head -- 2 k-subtiles, 2x PE throughput
                        for hp in range(2):
                            h = 2 * H + hp
                            nc.tensor.matmul(
                                ot[hp][:, 0:QB],
                                lhsT=vaug8[:, 2 * tpi:2 * tpi + 2,
                                           128 * h:128 * (h + 1)],
                                rhs=e8t[:, :, hp, :],
                                start=(tpi == 0), stop=False,
                                perf_mode=DROW)

                    def s_mm(ktile, c0, dst):
                        # dst: [128, 2(hp), 512] psum slice target
                        for hp in range(2):
                            h = 2 * H + hp
                            r0 = (HD * h) % 128
                            mi = (HD * h) // 128
                            nc.tensor.matmul(
                                dst[:, hp, c0:512],
                                lhsT=kt[r0:r0 + HD, mi,
                                        ktile * 128:(ktile + 1) * 128],
                                rhs=qt[r0:r0 + HD, mi, q0 + c0:q0 + QB],
                                start=True, stop=True)

                    pend = None     # PV lags so it never waits on exp
                    # off-diagonal ktiles in fp8 DoubleRow pairs
                    for tpi in range(2 * j):
                        e8t = ep.tile([128, 2, 2, QB], F8, name="e8", tag="e")
                        for sub in range(2):
                            ktile = 2 * tpi + sub
                            st = sp.tile([128, 2, 512], F32, name="st",
                                         tag="st")
                            s_mm(ktile, 0, st)
                            if sub == 0:
                                nc.scalar.activation(
                                    out=e8t[:, sub], in_=st, func=EXP,
                                    scale=SSCL, bias=lnkc[:, 0:1])
                            else:
                                # polynomial k*exp on DVE: runs concurrently
                                # with the ACT exp of sub 0
                                nc.vector._custom_dve(
                                    EXP4, out=e8t[:, sub], in0=st,
                                    s0=FA, s1=FB, imm2=FC)
                            fill_fn(j, H, ktile)
                        if pend is not None:
                            pend[0](*pend[1])
                        pend = (pv_pair, (tpi, e8t))
                    # diagonal ktiles: bf16 path with c0 skipping + tri mask
                    for s in range(4):
                        ktile = 4 * j + s
                        c0 = 128 * s
                        st = sp.tile([128, 2, 512], F32, name="st", tag="st")
                        e_t = ep.tile([128, 2, QB], BF16, name="e", tag="e")
                        s_mm(ktile, c0, st)
                        # only cols [c0:512) are computed/read --
                        # skip the stale region in the exp too
                        nc.scalar.activation(
                            out=e_t[:, :, c0:512], in_=st[:, :, c0:512],
                            func=EXP, scale=SSCL, bias=lnkc[:, 0:1])
                        # mask both heads' diag tile in one strided op;
                        # gpsimd keeps it off the busier DVE queue
                        dg = e_t[:, :, c0:c0 + 128]
                        trb = bass.AP(
                            tensor=tri.tensor, offset=tri.offset,
                            ap=[tri.ap[0], [0, 2], [1, 128]])
                        nc.gpsimd.tensor_mul(dg, dg, trb)
                        fill_fn(j, H, ktile)
                        if pend is not None:
                            pend[0](*pend[1])
                        pend = (pv_diag, (ktile, c0, e_t))
                    pend[0](*pend[1])
                    for hp in range(2):
                        h = 2 * H + hp
                        rec = bp.tile([64, QB], F32, name="rec", tag="rec")
                        nc.vector.reciprocal_approx_fast(rec, ot[hp][0:64, :])
                        r0 = (HD * h) % 128
                        mi = (HD * h) // 128
                        nc.vector.tensor_mul(
                            ot_all[r0:r0 + HD, mi, q0:q0 + QB],
                            ot[hp][64:128, :], rec)
                    if end_h is not None:
                        end_h(H)

            def gen_wo3a():
                # wo(3) kk=0 half: depends only on H0's ot_all -- runs as
                # H1 filler; partials parked in SBUF bf16
                q0 = (NJ - 1) * QB
                for qq in range(4):
                    row = q0 + qq * 128
                    for n in range(2):
                        ps = mp.tile([128, 512], F32, name="wo_ps", tag="mp")
                        nc.tensor.matmul(
                            ps, lhsT=ot_all[:, 0, row:row + 128],
                            rhs=wot[:, 0, n * 512:(n + 1) * 512],
                            start=True, stop=True)
                        nc.vector.tensor_copy(
                            wo3pb[:, qq, n * 512:(n + 1) * 512], ps)
                        yield

            def gen_wo3b():
                # wo(3) kk=1 half + add + streamed output
                q0 = (NJ - 1) * QB
                for qq in range(4):
                    row = q0 + qq * 128
                    ob = op.tile([128, DM], BF16, name="ob3", tag="ob")
                    for n in range(2):
                        ps = mp.tile([128, 512], F32, name="wo_ps", tag="mp")
                        nc.tensor.matmul(
                            ps, lhsT=ot_all[:, 1, row:row + 128],
                            rhs=wot[:, 1, n * 512:(n + 1) * 512],
                            start=True, stop=True)
                        nc.vector.tensor_add(
                            ob[:, n * 512:(n + 1) * 512], ps,
                            wo3pb[:, qq, n * 512:(n + 1) * 512])
                        yield
                    dma_eng = nc.sync if qq % 2 == 0 else nc.scalar
                    dma_eng.dma_start(out=p_out[row:row + 128, :], in_=ob)

            # ---- schedule ----
            # Round 0 prefix: Q m0, K m0, V g=0 -> attention(0) H0 can start;
            # the rest of proj(0) rides along as round-0 filler.
            for g in (proj_qk_m(wqt, bqc, qt, 0, 0, QDSC),
                      proj_qk_m(wkt, bkc, kt, 0, 0, KDSC),
                      proj_v_tt(0)):
                for _ in g:
                    pass
            local = deque()
            # m1 projections first (needed by attention(0) H1; always
            # runnable), then V t-subtiles in DMA-arrival order
            for g in (proj_qk_m(wqt, bqc, qt, 0, 1, QDSC, chunk=2),
                      proj_qk_m(wkt, bkc, kt, 0, 1, KDSC, chunk=2),
                      proj_v_tt(1), proj_v_tt(2), proj_v_tt(3)):
                local.append(("r0", g))

            def fill_r0(j, H, ktile):
                n = 3
                while n > 0 and local:
                    try:
                        next(local[0][1])
                        n -= 1
                    except StopIteration:
                        local.popleft()
                if not local:
                    fill(1)

            for ts in range(1, NJ):
                fillers.append((("qk", ts), gen_proj_qk(ts)))
                fillers.append((("v", ts), gen_proj_v(ts)))

            def fill_main(j, H, ktile):
                if H == 0 and ktile == 4 * j:
                    force(("v", j))   # vaug t-slice j gate (usually a no-op)
                # the PE queue is in-order: drain enough each step that
                # late-emitted wo work interleaves into round-3 stalls
                fill(2 if j < 3 else 3)

            attention(0, fill_r0)
            while local:
                fill_r0(0, 0, 0)
            def queue_wo3a(H):
                if H == 0:
                    fillers.append((("wo3a",), gen_wo3a()))

            for j in range(1, NJ):
                force(("qk", j))
                if j == NJ - 1:
                    # wo is the only filler left whose deadline is the kernel
                    # end; spend it on the most ACT-bound round.
                    for jj in range(NJ - 1):
                        fillers.append((("wo", jj), gen_wo(jj)))
                attention(j, fill_main,
                          end_h=queue_wo3a if j == NJ - 1 else None)
            while fillers:
                fill(64)
            for _ in gen_wo3b():
                pass
    nc.compile()
    return nc


def _prep_inputs(x, wq, bq, wk, bk, wv, wo):
    bf = ml_dtypes.bfloat16
    f8 = ml_dtypes.float8_e4m3fn
    scale = np.float32(1.0 / np.sqrt(HD))
    tri = np.triu(np.ones((128, 128), np.float32)).astype(bf)
    in_maps = []
    for c in range(NCORES):
        b, g = c // 4, c % 4
        sl = slice(DS * g, DS * (g + 1))
        xts = np.ascontiguousarray(
            x[b].T.astype(bf).reshape(8, 128, 4, 512).transpose(2, 1, 0, 3))
        in_maps.append({
            "xt": xts,
            "xt8": xts.astype(f8),
            "wqt": np.ascontiguousarray(
                wq.T[:, sl] * (scale / SSCL * WSCL_Q)).astype(f8),
            "wkt": np.ascontiguousarray(wk.T[:, sl] * WSCL_K).astype(f8),
            "wvt": np.ascontiguousarray(wv.T[:, sl]).astype(bf),
            "wot": np.ascontiguousarray(wo[:, sl].T).astype(bf),
            "bqc": np.ascontiguousarray(
                (bq[sl] * scale / SSCL).reshape(2, 128).T).astype(np.float32),
            "bkc": np.ascontiguousarray(
                bk[sl].reshape(2, 128).T).astype(np.float32),
            "tri": tri,
        })
    return in_maps


TRACE = False
TRACE_DIR = None
LAST_RESULT = None


def kernel(x, wq, bq, wk, bk, wv, bv, wo, bo):
    global LAST_RESULT
    x, wq, bq, wk, bk, wv, bv, wo, bo = [
        np.asarray(a, np.float32)
        for a in (x, wq, bq, wk, bk, wv, bv, wo, bo)]
    if "nc" not in _CACHE:
        _CACHE["nc"] = _build()
    nc = _CACHE["nc"]
    in_maps = _prep_inputs(x, wq, bq, wk, bk, wv, wo)
    res = bass_utils.run_bass_kernel_spmd(
        nc, in_maps, core_ids=list(range(NCORES)), trace=TRACE,
        tmpdir=TRACE_DIR)
    LAST_RESULT = res
    # bv commutes through softmax (rows sum to 1): out += bv @ wo^T + bo.
    const_row = (bv.astype(np.float64) @ wo.T.astype(np.float64) +
                 bo.astype(np.float64)).astype(np.float32)
    out = np.empty((2, T, DM), np.float32)
    for b in range(2):
        acc = res.results[4 * b]["p_out"].astype(np.float32)
        for g in range(1, 4):
            acc = acc + res.results[4 * b + g]["p_out"].astype(np.float32)
        out[b] = acc + const_row
    return out



# revision 58
# speedup vs baseline: 1.2984x; 1.2984x over previous
"""Multi-head self-attention (B=2, T=2048, d_model=1024, 16 heads, causal)
on 8 trn2 NeuronCores.

Sharding: core c -> batch b=c//4, head-group g=c%4 (4 heads, d_model slice
of 256). Each core computes its heads' attention and a partial wo
projection [2048, 1024] (bf16); host sums the 4 partials per batch and
adds bo + bv @ wo^T (the V-bias commutes through softmax since rows sum
to 1, so it is a host-side constant).

Per-core pipeline (all matmul inputs bf16, fp32 PSUM accumulation):
  QT[dq,t] = (wq_s @ x^T)*0.125 + bq*0.125   (scale folded into weights)
  KT[dk,t] = wk_s @ x^T + bk
  V[t,dv]  = x @ wv_s^T
  per head pair, per q-block j (512 wide):
    S^T[k,q] = K_h @ Q_h^T   (K=64 contraction, heads packed at partition
                              bases 0/64 -> concurrent row-group matmuls)
    E = exp(S^T)             (ACT, one call per (head-pair, ktile))
    diag tiles: E *= upper-tri mask
    O^T_aug = V_aug^T @ E    (V_aug = [64 ones cols | V_h cols] so PSUM rows
                              0-63 = replicated rowsums, rows 64-127 = O^T)
    OT_all = O^T * recip(rowsum)  (DVE reciprocal + tensor_tensor mul)
  P = OT_all^T @ wo_s^T      (partial output, bf16 out)

Scheduling: the per-ktile chain S(PE) -> exp(ACT) -> PV(PE) leaves PE
under-filled during attention (ACT needs ~1.15us per ktile vs ~0.64us of
attention PE work).  Projection and wo matmuls for neighboring q-blocks
are emitted as generator-based filler chunks (~0.4us of PE work per
chunk) interleaved between attention ktile steps; wo work is held back
to round 3, the most ACT-bound stretch.  x is staged t-slice-major
(both DRAM and SBUF) so each input DMA moves 8KB-contiguous lines per
partition at full rate and projections of t-slice 0 start ~10us in.
"""
import sys
sys.path.insert(0, "/opt/trn_rl_repo")
from collections import deque

import numpy as np
import ml_dtypes

import concourse.bass as bass
import concourse.bacc as bacc
import concourse.tile as tile
import concourse.mybir as mybir
from concourse import bass_utils

BF16 = mybir.dt.bfloat16
F32 = mybir.dt.float32
F8 = mybir.dt.float8e4
EXP = mybir.ActivationFunctionType.Exp
DROW = mybir.MatmulPerfMode.DoubleRow

T = 2048          # sequence length
DM = 1024         # d_model
DS = 256          # per-core d_model slice (4 heads x 64)
HD = 64           # head dim
NH = 4            # heads per core
KT128 = 16        # k tiles of 128 over T
QB = 512          # q block width
NJ = T // QB      # 4 q blocks
NCORES = 8
# Scores are pre-scaled by 1/SSCL (folded into wq with the 1/sqrt(64)):
# ACT computes k*exp(score) via exp(t*SSCL + LNK); the DVE EXP4 custom op
# computes the same k*exp(score) as (((t+FA)t+FB)((t+FC)t+1))^2.  The
# common factor k cancels in the softmax normalization.
SSCL = 5.011862351873014
FA = 0.5023871747892565
FB = 0.8155188024921279
FC = 1.9177204582091354
LNK = -0.3906937361657684
WSCL_Q = 1024.0   # fp8 pre-scale on wq.T*0.125/SSCL (std 5e-4 -> 0.51)
WSCL_K = 32.0     # fp8 pre-scale on wk.T (std 0.02 -> 0.64)
QDSC = 1.0 / WSCL_Q
KDSC = 1.0 / WSCL_K


def _register_exp4():
    from concourse import dve_ops as DO
    from concourse.dve_spec import Spec, Src0, C0, C1, C2, One, sq, lower
    from concourse.dve_uop import DveOpSpec

    if "EXP4_ANT" in DO._SUB_OPCODE_FOR_NAME:
        return DO.OPS[DO._SUB_OPCODE_FOR_NAME["EXP4_ANT"]
                      - DO._CUSTOM_DVE_ROW_BASE]
    x = Src0
    body = sq(((x + C0) * x + C1) * ((x + C2) * x + One))

    def ref(in0, in1, s0, s1, imm2):
        xr = in0.astype(np.float32)
        p = ((xr + s0) * xr + s1) * ((xr + imm2) * xr + 1.0)
        return (p * p).astype(np.float32)

    spec = Spec(body=body, reference=ref)
    row = DO._CUSTOM_DVE_ROW_BASE + len(DO.OPS)
    shas = {}
    for ver in ("v3", "v4"):
        uops = lower(spec, ver=ver)
        shas[ver] = DveOpSpec(name="EXP4_ANT", opcode=row, uops=uops,
                              rd1_en=False).sha(ver)
    op = DO.DveOp("EXP4_ANT", spec, subdim=False, uops_sha=shas)
    DO.OPS.append(op)
    DO._SUB_OPCODE_FOR_NAME["EXP4_ANT"] = row
    DO.CUSTOM_DVE_SPECS["EXP4_ANT"] = spec
    return op


EXP4 = _register_exp4()

_CACHE = {}


def _build():
    nc = bacc.Bacc("TRN2", target_bir_lowering=False, debug=False,
                   enable_asserts=False, num_devices=NCORES)
    dram = {}
    for name, shape, dt in [
        ("xt", [NJ, 128, 8, 512], BF16),  # x[b]^T, t-slice-major chunks
        ("xt8", [NJ, 128, 8, 512], F8),   # same, fp8 (QK-proj rhs)
        ("wqt", [DM, DS], F8),        # wq.T[:, slice] * 0.125/SSCL * 1024
        ("wkt", [DM, DS], F8),        # wk.T[:, slice] * 32 (fp8)
        ("wvt", [DM, DS], BF16),
        ("wot", [DS, DM], BF16),      # wo[:, slice].T
        ("bqc", [128, 2], F32),       # bq*0.125 as [128, m] columns
        ("bkc", [128, 2], F32),
        ("tri", [128, 128], BF16),    # upper-tri (incl diag) ones
    ]:
        dram[name] = nc.dram_tensor(name, shape, dt, kind="ExternalInput").ap()
    p_out = nc.dram_tensor("p_out", [T, DM], BF16, kind="ExternalOutput").ap()

    with tile.TileContext(nc) as tc:
        with tc.tile_pool(name="persist", bufs=1) as pp, \
             tc.tile_pool(name="epool", bufs=4) as ep, \
             tc.tile_pool(name="outp", bufs=2) as op, \
             tc.tile_pool(name="bcp", bufs=2) as bp, \
             tc.tile_pool(name="misc_ps", bufs=2, space="PSUM") as mp, \
             tc.tile_pool(name="st_ps", bufs=2, space="PSUM") as sp, \
             tc.tile_pool(name="ot_ps", bufs=1, space="PSUM") as tp:

            # ---- persistent SBUF ----
            xt = pp.tile([128, NJ, 8, 512], BF16, name="xt")  # [p, ts, kt8, c]
            xt8 = pp.tile([128, NJ, 8, 512], F8, name="xt8")  # fp8 copy (QK)
            wqt = pp.tile([128, 8, DS], F8, name="wqt")
            wkt = pp.tile([128, 8, DS], F8, name="wkt")
            wvt = pp.tile([128, 8, DS], BF16, name="wvt")
            wot = pp.tile([128, 2, DM], BF16, name="wot")
            qt = pp.tile([128, 2, T], BF16, name="qt")        # [p, dq-tile, t]
            kt = pp.tile([128, 2, T], BF16, name="kt")
            vaug = pp.tile([128, KT128, 512], BF16, name="vaug")
            vaug8 = pp.tile([128, KT128, 512], F8, name="vaug8")
            ot_all = pp.tile([128, 2, T], BF16, name="ot_all")
            wo3pb = pp.tile([128, 4, DM], BF16, name="wo3pb")
            ones_row = pp.tile([1, 512], BF16, name="ones_row")
            bqc = pp.tile([128, 2], F32, name="bqc")
            bkc = pp.tile([128, 2], F32, name="bkc")
            tri = pp.tile([128, 128], BF16, name="tri")
            lnkc = pp.tile([128, 1], F32, name="lnkc")

            # ones_row feeds the PE warmup; DVE is otherwise idle at start.
            nc.vector.memset(ones_row, 1.0)
            nc.vector.memset(lnkc, LNK)
            # PE warmup: dummy matmuls on scratch during the input DMA wait so
            # HAM is at full clock when real matmuls start (results unread)
            warm = mp.tile([128, 512], F32, name="warm", tag="mp")
            # bridges the input-DMA window at the throttled clock so HAM
            # never sees a >3.4us PE idle and proj(0) starts at full clock
            for _ in range(8):
                nc.tensor.matmul(warm, lhsT=ones_row[0:1, 0:128],
                                 rhs=ones_row[0:1, :], start=True, stop=True)

            # Input DMA. A single queue sustains only ~100GB/s, so the head
            # load is split in priority order across the three trigger-capable
            # engines (sync/scalar/gpsimd), in chunks small enough that the
            # proj(0) contraction can start while the rest streams in.
            def w_dram(nm):
                return dram[nm].rearrange("(kt p) d -> p kt d", p=128)

            def w_dram_half(nm, m):
                return dram[nm].rearrange(
                    "(kt p) d -> p kt d", p=128)[:, :, 128 * m:128 * (m + 1)]

            wq_d, wk_d, wv_d, wo_d = (w_dram(n) for n in
                                      ("wqt", "wkt", "wvt", "wot"))
            # Everything needed in the first ~25us rides the fast sync
            # (HWDGE) queue in need-order -- the scalar/gpsimd SWDGE queues
            # only sustain ~30GB/s, so they carry the late-needed bulk:
            # x_2 (attention(2) diag), x8_3/x_3/wot (round 3).
            # The DMA queue round-robins packets across every queued
            # transfer, so completion order ~= trigger order only when a
            # queue is lightly loaded.  Critical path (wqt -> xt8_0 -> x_0)
            # leads sync; tiny-row bqc/bkc (8B rows, trigger-dominated)
            # lead scalar; late-needed bulk rides gpsimd.
            nc.sync.dma_start(out=wqt[:, :, 0:128], in_=w_dram_half("wqt", 0))
            nc.sync.dma_start(out=xt8[:, 0, 0:4], in_=dram["xt8"][0, :, 0:4])
            nc.sync.dma_start(out=wqt[:, :, 128:256], in_=w_dram_half("wqt", 1))
            nc.sync.dma_start(out=xt8[:, 0, 4:8], in_=dram["xt8"][0, :, 4:8])
            nc.sync.dma_start(out=xt[:, 0, 0:4], in_=dram["xt"][0, :, 0:4])
            nc.sync.dma_start(out=xt[:, 0, 4:8], in_=dram["xt"][0, :, 4:8])
            nc.sync.dma_start(out=wvt, in_=wv_d)
            nc.sync.dma_start(
                out=xt8[:, 1:4],
                in_=dram["xt8"][1:4].rearrange("ts p kt c -> p ts kt c"))
            nc.sync.dma_start(
                out=xt[:, 1:4],
                in_=dram["xt"][1:4].rearrange("ts p kt c -> p ts kt c"))
            nc.scalar.dma_start(out=bqc, in_=dram["bqc"])
            nc.scalar.dma_start(out=bkc, in_=dram["bkc"])
            nc.scalar.dma_start(out=wkt, in_=wk_d)
            nc.scalar.dma_start(out=tri, in_=dram["tri"])
            nc.gpsimd.dma_start(out=wot, in_=wo_d)
            # V_aug head block h: cols [128h, 128h+64) ones, [128h+64, +128) V.
            # After the DMA triggers so they don't delay the weight loads.
            for h in range(NH):
                nc.gpsimd.memset(vaug[:, :, 128 * h:128 * h + HD], 1.0)
                nc.gpsimd.memset(vaug8[:, :, 128 * h:128 * h + HD], 1.0)

            # ---- emission helpers ----
            def proj_qk_m(w_sb, b_c, dst, ts, m, descale, chunk=99):
                """One [128,512] psum tile of the Q/K projection (fp8
                DoubleRow, 2 k-subtiles per matmul); yields every `chunk`
                matmuls. The fp8 weight pre-scale is undone via `descale`
                in the bias stage."""
                t0 = ts * 512
                ps = mp.tile([128, 512], F32, name="proj_ps", tag="mp")
                for k in range(4):
                    nc.tensor.matmul(
                        ps, lhsT=w_sb[:, 2 * k:2 * k + 2,
                                      m * 128:(m + 1) * 128],
                        rhs=xt8[:, ts, 2 * k:2 * k + 2, :],
                        start=(k == 0), stop=(k == 3), perf_mode=DROW)
                    if k % chunk == chunk - 1 and k < 3:
                        yield
                if ts < 3:
                    # ACT has slack in the early rounds and reads PSUM fast;
                    # this also frees the psum slot sooner than the busier DVE
                    nc.scalar.activation(
                        out=dst[:, m, t0:t0 + 512], in_=ps,
                        func=mybir.ActivationFunctionType.Identity,
                        bias=b_c[:, m:m + 1], scale=descale)
                else:
                    nc.vector.tensor_scalar(
                        dst[:, m, t0:t0 + 512], ps, descale,
                        b_c[:, m:m + 1], mybir.AluOpType.mult,
                        mybir.AluOpType.add)
                yield

            def proj_v_tt(g):
                """V projection for t-subtile g (128 rows): 8 matmuls + the
                scatter into vaug."""
                ps = mp.tile([128, 256], F32, name="v_ps", tag="mp")
                for k in range(8):
                    nc.tensor.matmul(
                        ps, lhsT=xt[:, g // 4, k, (g % 4) * 128:(g % 4 + 1) * 128],
                        rhs=wvt[:, k, :], start=(k == 0), stop=(k == 7))
                    if k == 3:
                        yield
                # scatter into vaug (bf16 diag path + fp8 off-diag path):
                # head h -> cols [128h+64, 128h+128)
                nc.vector.tensor_copy(
                    vaug[:, g, :].rearrange("p (h c) -> p h c", h=NH)[:, :, HD:],
                    ps.rearrange("p (h c) -> p h c", h=NH))
                # the fp8 copy derives from the bf16 one in SBUF on ACT
                # (gpsimd can't read PSUM and queues ahead of diag masks;
                # DVE is the most loaded engine)
                nc.scalar.activation(
                    out=vaug8[:, g, :].rearrange(
                        "p (h c) -> p h c", h=NH)[:, :, HD:],
                    in_=vaug[:, g, :].rearrange(
                        "p (h c) -> p h c", h=NH)[:, :, HD:],
                    func=mybir.ActivationFunctionType.Copy)
                yield

            def gen_proj_qk(ts):
                for w_sb, b_c, dst, dsc in ((wqt, bqc, qt, QDSC),
                                            (wkt, bkc, kt, KDSC)):
                    for m in range(2):
                        yield from proj_qk_m(w_sb, b_c, dst, ts, m, dsc,
                                             chunk=1)

            def gen_proj_v(ts):
                for tt in range(4):
                    yield from proj_v_tt(4 * ts + tt)

            def gen_wo(j):
                q0 = j * QB
                last = j == NJ - 1
                ob = op.tile([128, 4, DM], BF16, name="ob", tag="ob")
                for qq in range(4):
                    row = q0 + qq * 128
                    for n in range(2):
                        ps = mp.tile([128, 512], F32, name="wo_ps", tag="mp")
                        for kk in range(2):
                            nc.tensor.matmul(
                                ps, lhsT=ot_all[:, kk, row:row + 128],
                                rhs=wot[:, kk, n * 512:(n + 1) * 512],
                                start=(kk == 0), stop=(kk == 1))
                        # split the psum->sbuf casts between DVE and ACT so
                        # neither engine stalls the attention exp chain
                        if n == 0:
                            nc.vector.tensor_copy(
                                ob[:, qq, n * 512:(n + 1) * 512], ps)
                        else:
                            nc.scalar.activation(
                                out=ob[:, qq, n * 512:(n + 1) * 512], in_=ps,
                                func=mybir.ActivationFunctionType.Identity)
                        yield
                    if last:
                        # tail round: stream the output per 128-row chunk so
                        # the final DMA only trails the last copy (HWDGE
                        # queues only -- gpsimd's SWDGE has slow start)
                        dma_eng = nc.sync if qq % 2 == 0 else nc.scalar
                        dma_eng.dma_start(
                            out=p_out[row:row + 128, :], in_=ob[:, qq, :])
                if not last:
                    dma_eng = nc.sync if j % 2 == 0 else nc.scalar
                    dma_eng.dma_start(
                        out=p_out[q0:q0 + 512, :].rearrange(
                            "(q p) d -> p q d", p=128),
                        in_=ob)
                yield

            # Global filler queue: (key, generator), drained strictly FIFO.
            fillers = deque()

            def fill(n):
                while n > 0 and fillers:
                    try:
                        next(fillers[0][1])
                        n -= 1
                    except StopIteration:
                        fillers.popleft()

            def force(key):
                """Drain fillers until generator `key` has been exhausted."""
                while any(k == key for k, _ in fillers):
                    try:
                        next(fillers[0][1])
                    except StopIteration:
                        fillers.popleft()

            def attention(j, fill_fn, end_h=None):
                q0 = j * QB
                nk = 4 * (j + 1)           # k-tiles of 128
                for H in range(2):          # head pair (2H, 2H+1)
                    ot = [tp.tile([128, QB], F32, name=f"ot{hp}", tag=f"ot{hp}")
                          for hp in range(2)]

                    def pv_diag(ktile, c0, e_t):
                        for hp in range(2):
                            h = 2 * H + hp
                            nc.tensor.matmul(
                                ot[hp][:, c0:QB],
                                lhsT=vaug[:, ktile, 128 * h:128 * (h + 1)],
                                rhs=e_t[:, hp, c0:QB],
                                start=(ktile == 0), stop=(ktile == nk - 1))

                    def pv_pair(tpi, e8t):
                        # fp8 DoubleRow: ktiles (2tpi, 2tpi+1) in one matmul
                        # per head -- 2 k-subtiles, 2x PE throughput
                        for hp in range(2):
                            h = 2 * H + hp
                            nc.tensor.matmul(
                                ot[hp][:, 0:QB],
                                lhsT=vaug8[:, 2 * tpi:2 * tpi + 2,
                                           128 * h:128 * (h + 1)],
                                rhs=e8t[:, :, hp, :],
                                start=(tpi == 0), stop=False,
                                perf_mode=DROW)

                    def s_mm(ktile, c0, dst):
                        # dst: [128, 2(hp), 512] psum slice target
                        for hp in range(2):
                            h = 2 * H + hp
                            r0 = (HD * h) % 128
                            mi = (HD * h) // 128
                            nc.tensor.matmul(
                                dst[:, hp, c0:512],
                                lhsT=kt[r0:r0 + HD, mi,
                                        ktile * 128:(ktile + 1) * 128],
                                rhs=qt[r0:r0 + HD, mi, q0 + c0:q0 + QB],
                                start=True, stop=True)

                    pend = None     # PV lags so it never waits on exp
                    # off-diagonal ktiles in fp8 DoubleRow pairs
                    for tpi in range(2 * j):
                        e8t = ep.tile([128, 2, 2, QB], F8, name="e8", tag="e")
                        for sub in range(2):
                            ktile = 2 * tpi + sub
                            st = sp.tile([128, 2, 512], F32, name="st",
                                         tag="st")
                            s_mm(ktile, 0, st)
                            if sub == 0:
                                nc.scalar.activation(
                                    out=e8t[:, sub], in_=st, func=EXP,
                                    scale=SSCL, bias=lnkc[:, 0:1])
                            else:
                                # polynomial k*exp on DVE: runs concurrently
                                # with the ACT exp of sub 0
                                nc.vector._custom_dve(
                                    EXP4, out=e8t[:, sub], in0=st,
                                    s0=FA, s1=FB, imm2=FC)
                            fill_fn(j, H, ktile)
                        if pend is not None:
                            pend[0](*pend[1])
                        pend = (pv_pair, (tpi, e8t))
                    # diagonal ktiles: bf16 path with c0 skipping + tri mask
                    for s in range(4):
                        ktile = 4 * j + s
                        c0 = 128 * s
                        st = sp.tile([128, 2, 512], F32, name="st", tag="st")
                        e_t = ep.tile([128, 2, QB], BF16, name="e", tag="e")
                        s_mm(ktile, c0, st)
                        # only cols [c0:512) are computed/read --
                        # skip the stale region in the exp too
                        nc.scalar.activation(
                            out=e_t[:, :, c0:512], in_=st[:, :, c0:512],
                            func=EXP, scale=SSCL, bias=lnkc[:, 0:1])
                        # mask both heads' diag tile in one strided op;
                        # gpsimd keeps it off the busier DVE queue
                        dg = e_t[:, :, c0:c0 + 128]
                        trb = bass.AP(
                            tensor=tri.tensor, offset=tri.offset,
                            ap=[tri.ap[0], [0, 2], [1, 128]])
                        nc.gpsimd.tensor_mul(dg, dg, trb)
                        fill_fn(j, H, ktile)
                        if pend is not None:
                            pend[0](*pend[1])
                        pend = (pv_diag, (ktile, c0, e_t))
                    pend[0](*pend[1])
                    for hp in range(2):
                        h = 2 * H + hp
                        rec = bp.tile([64, QB], F32, name="rec", tag="rec")
                        nc.vector.reciprocal_approx_fast(rec, ot[hp][0:64, :])
                        r0 = (HD * h) % 128
                        mi = (HD * h) // 128
                        nc.vector.tensor_mul(
                            ot_all[r0:r0 + HD, mi, q0:q0 + QB],
                            ot[hp][64:128, :], rec)
                    if end_h is not None:
                        end_h(H)

            def gen_wo3a():
                # wo(3) kk=0 half: depends only on H0's ot_all -- runs as
                # H1 filler; partials parked in SBUF bf16
                q0 = (NJ - 1) * QB
                for qq in range(4):
                    row = q0 + qq * 128
                    for n in range(2):
                        ps = mp.tile([128, 512], F32, name="wo_ps", tag="mp")
                        nc.tensor.matmul(
                            ps, lhsT=ot_all[:, 0, row:row + 128],
                            rhs=wot[:, 0, n * 512:(n + 1) * 512],
                            start=True, stop=True)
                        nc.vector.tensor_copy(
                            wo3pb[:, qq, n * 512:(n + 1) * 512], ps)
                        yield

            def gen_wo3b():
                # wo(3) kk=1 half + add + streamed output
                q0 = (NJ - 1) * QB
                for qq in range(4):
                    row = q0 + qq * 128
                    ob = op.tile([128, DM], BF16, name="ob3", tag="ob")
                    for n in range(2):
                        ps = mp.tile([128, 512], F32, name="wo_ps", tag="mp")
                        nc.tensor.matmul(
                            ps, lhsT=ot_all[:, 1, row:row + 128],
                            rhs=wot[:, 1, n * 512:(n + 1) * 512],
                            start=True, stop=True)
                        nc.vector.tensor_add(
                            ob[:, n * 512:(n + 1) * 512], ps,
                            wo3pb[:, qq, n * 512:(n + 1) * 512])
                        yield
                    dma_eng = nc.sync if qq % 2 == 0 else nc.scalar
                    dma_eng.dma_start(out=p_out[row:row + 128, :], in_=ob)

            # ---- schedule ----
            # Round 0 prefix: Q m0, K m0, V g=0 -> attention(0) H0 can start;
            # the rest of proj(0) rides along as round-0 filler.
            for g in (proj_qk_m(wqt, bqc, qt, 0, 0, QDSC),
                      proj_qk_m(wkt, bkc, kt, 0, 0, KDSC),
                      proj_v_tt(0)):
                for _ in g:
                    pass
            local = deque()
            # m1 projections first (needed by attention(0) H1; always
            # runnable), then V t-subtiles in DMA-arrival order
            for g in (proj_qk_m(wqt, bqc, qt, 0, 1, QDSC, chunk=2),
                      proj_qk_m(wkt, bkc, kt, 0, 1, KDSC, chunk=2),
                      proj_v_tt(1), proj_v_tt(2), proj_v_tt(3)):
                local.append(("r0", g))

            def fill_r0(j, H, ktile):
                n = 3
                while n > 0 and local:
                    try:
                        next(local[0][1])
                        n -= 1
                    except StopIteration:
                        local.popleft()
                if not local:
                    fill(1)

            for ts in range(1, NJ):
                fillers.append((("qk", ts), gen_proj_qk(ts)))
                fillers.append((("v", ts), gen_proj_v(ts)))

            def fill_main(j, H, ktile):
                if H == 0 and ktile == 4 * j:
                    force(("v", j))   # vaug t-slice j gate (usually a no-op)
                # deeper drain early keeps the PE queue primed; round 3
                # holds back so the wo fillers last through both halves
                fill(2 if j < 3 else 1)

            attention(0, fill_r0)
            while local:
                fill_r0(0, 0, 0)
            def queue_wo3a(H):
                if H == 0:
                    fillers.append((("wo3a",), gen_wo3a()))

            for j in range(1, NJ):
                force(("qk", j))
                if j == NJ - 1:
                    # wo is the only filler left whose deadline is the kernel
                    # end; spend it on the most ACT-bound round.
                    for jj in range(NJ - 1):
                        fillers.append((("wo", jj), gen_wo(jj)))
                attention(j, fill_main,
                          end_h=queue_wo3a if j == NJ - 1 else None)
            while fillers:
                fill(64)
            for _ in gen_wo3b():
                pass
    nc.compile()
    return nc


def _prep_inputs(x, wq, bq, wk, bk, wv, wo):
    bf = ml_dtypes.bfloat16
    f8 = ml_dtypes.float8_e4m3fn
    scale = np.float32(1.0 / np.sqrt(HD))
    tri = np.triu(np.ones((128, 128), np.float32)).astype(bf)
    in_maps = []
    for c in range(NCORES):
        b, g = c // 4, c % 4
        sl = slice(DS * g, DS * (g + 1))
        xts = np.ascontiguousarray(
            x[b].T.astype(bf).reshape(8, 128, 4, 512).transpose(2, 1, 0, 3))
        in_maps.append({
            "xt": xts,
            "xt8": xts.astype(f8),
            "wqt": np.ascontiguousarray(
                wq.T[:, sl] * (scale / SSCL * WSCL_Q)).astype(f8),
            "wkt": np.ascontiguousarray(wk.T[:, sl] * WSCL_K).astype(f8),
            "wvt": np.ascontiguousarray(wv.T[:, sl]).astype(bf),
            "wot": np.ascontiguousarray(wo[:, sl].T).astype(bf),
            "bqc": np.ascontiguousarray(
                (bq[sl] * scale / SSCL).reshape(2, 128).T).astype(np.float32),
            "bkc": np.ascontiguousarray(
                bk[sl].reshape(2, 128).T).astype(np.float32),
            "tri": tri,
        })
    return in_maps


TRACE = False
TRACE_DIR = None
LAST_RESULT = None


def kernel(x, wq, bq, wk, bk, wv, bv, wo, bo):
    global LAST_RESULT
    x, wq, bq, wk, bk, wv, bv, wo, bo = [
        np.asarray(a, np.float32)
        for a in (x, wq, bq, wk, bk, wv, bv, wo, bo)]
    if "nc" not in _CACHE:
        _CACHE["nc"] = _build()
    nc = _CACHE["nc"]
    in_maps = _prep_inputs(x, wq, bq, wk, bk, wv, wo)
    res = bass_utils.run_bass_kernel_spmd(
        nc, in_maps, core_ids=list(range(NCORES)), trace=TRACE,
        tmpdir=TRACE_DIR)
    LAST_RESULT = res
    # bv commutes through softmax (rows sum to 1): out += bv @ wo^T + bo.
    const_row = (bv.astype(np.float64) @ wo.T.astype(np.float64) +
                 bo.astype(np.float64)).astype(np.float32)
    out = np.empty((2, T, DM), np.float32)
    for b in range(2):
        acc = res.results[4 * b]["p_out"].astype(np.float32)
        for g in range(1, 4):
            acc = acc + res.results[4 * b + g]["p_out"].astype(np.float32)
        out[b] = acc + const_row
    return out

